# revision 18
# baseline (speedup 1.0000x reference)
"""Trainium2 Bass kernel for nn_FeatureEmbedding (4-layer 3x3 conv CNN
with LeakyReLU + sinusoidal positional-encoding add).

Strategy
--------
Data-parallel over the batch dim: 32 batches x 12 frames = 384 images;
each of the 8 NeuronCores processes 48 images (4 batches).

Per image the layer chain runs out of SBUF on zero-padded pitch-66
buffers ([C, 66*66], interiors rewritten per image, halos zeroed once):

  - Input arrives host-padded ([5, 66*66] per image, borders zero), so
    the 9 (kh, kw)-shifted copies that fold layer 1 into one K=45
    matmul per 512-pixel tile are 9 uniform contiguous DMAs with no
    clipping and no wrap-scrub memsets. A 10th SBUF->SBUF DMA
    duplicates the 45 partitions to partitions 64-108 so layer-1
    matmuls can be packed four-at-a-time onto disjoint 64x64 array
    quadrants via tile_position (row+col tiling: ~4x concurrency for
    the K=45, M=64 layer-1 matmuls).
  - Layer 2 (Cin=64) is fully K-packed: 4 paired K=128 matmuls + 1
    single K=64 per tile. The pairs read [A | A-colshift] (h1 parts
    64-127 hold a one-column-left copy) and [A | A-rowshift] (h1c).
    The shifted copies are built per-tile by small DVE copies right
    after each layer-1 drain - no whole-buffer barrier DMA. The two
    K=64 singles of each 2-tile block are row-tiled onto array row
    groups 0-1 / 2-3 (the second one reads the colshift copy at a
    one-left window, which equals the original tap input), so they run
    concurrently.
  - Layers 3-4 are 9-tap shift-GEMM (K=128 per tap) over the padded
    buffers, full-window strided moving access patterns.
  - Matmuls are emitted in 2-tile blocks with tap-outer order inside.
    The front phase (L1+L2) and back phase (L3+L4) share two 4-bank
    PSUM pools with dynamic slot allocation, so PSUM-slot reuse never
    couples one image's layer-1 to another image's layer-4 drains (a
    single shared 8-bank pool was the main serialization in the
    previous version).
  - ScalarE drains every bank with fused Lrelu(psum + bias); layer 4
    drains to an f32 buffer, VectorE adds the per-(t, channel)
    positional-encoding scalar, DMA to DRAM.
  - Emission is software-pipelined: front(i) = x9 prefetch for i+1 +
    L1/L2 of image i; back(i-1) = L3/L4 + PE-add + store of image i-1.
    The Tile list scheduler fills any front-phase drain-latency stalls
    with ready back-phase matmuls, keeping PE dense (avoids both idle
    gaps and the 2x p-state ramp penalty after PE idle).

Inputs are bf16 (host RNE cast), fp32 accumulation in PSUM; weights
are pre-marshaled on the host into the [K, M] stationary layouts the
PE wants (layer-1 and layer-2-single weights duplicated across
partition halves for the array-tiling variants); the PE table (pure
function of shapes) is precomputed on the host and passed in.
"""

import numpy as np

import concourse.bass as bass
import concourse.bacc as bacc
import concourse.mybir as mybir
import concourse.tile as tile

F32 = mybir.dt.float32
BF16 = mybir.dt.bfloat16
AF = mybir.ActivationFunctionType

N_CORES = 8
B, T, CIN, H, W = 32, 12, 5, 64, 64
CH = [64, 128, 128, 128]
K1 = 45                # 9 taps * 5 cin folded into layer-1 contraction
NPIX = H * W           # 4096
PITCH = W + 2          # 66 (padded row pitch)
PAD = PITCH * PITCH    # 4356
NTILE = 8              # 512-pixel output tiles per image
RPT = H // NTILE       # 8 rows per tile
TILEPIX = RPT * W      # 512
X9W = H * PITCH        # 4224 (x9 free size; pitch-66 rows)
X9LEN = (H - 1) * PITCH + W  # 4222 (valid span per shifted copy)
ALPHA = 0.01           # LeakyReLU negative slope

TAPS = [(kh, kw) for kh in range(3) for kw in range(3)]
BLOCKT = 2             # tiles per tap-outer matmul block


def _build(nimg: int):
    """Build the per-core Bass program (SPMD: same program on all cores)."""
    nc = bacc.Bacc("TRN2", target_bir_lowering=False, debug=False)

    xin = nc.dram_tensor("xin", [nimg, CIN, PAD], BF16, kind="ExternalInput")
    w1d = nc.dram_tensor("w1", [2 * CH[0], CH[0]], BF16,
                         kind="ExternalInput")
    w2pd = nc.dram_tensor("w2p", [2 * CH[0], 3 * CH[1]], BF16,
                          kind="ExternalInput")
    w2cd = nc.dram_tensor("w2c", [2 * CH[0], CH[1]], BF16,
                          kind="ExternalInput")
    w2sd = nc.dram_tensor("w2s", [2 * CH[0], CH[1]], BF16,
                          kind="ExternalInput")
    w3d = nc.dram_tensor("w3", [CH[1], 9 * CH[2]], BF16, kind="ExternalInput")
    w4d = nc.dram_tensor("w4", [CH[2], 9 * CH[3]], BF16, kind="ExternalInput")
    b1d = nc.dram_tensor("b1", [CH[0], 1], F32, kind="ExternalInput")
    b2d = nc.dram_tensor("b2", [CH[1], 1], F32, kind="ExternalInput")
    b3d = nc.dram_tensor("b3", [CH[2], 1], F32, kind="ExternalInput")
    b4d = nc.dram_tensor("b4", [CH[3], 1], F32, kind="ExternalInput")
    ped = nc.dram_tensor("pe", [CH[3], T], F32, kind="ExternalInput")
    outd = nc.dram_tensor("out", [nimg, CH[3], NPIX], F32,
                          kind="ExternalOutput")

    with tile.TileContext(nc) as tc:
        with (
            tc.tile_pool(name="wpool", bufs=1) as wp,
            tc.tile_pool(name="bpool", bufs=1) as bp,
            tc.tile_pool(name="psumA", bufs=4, space="PSUM") as ppA,
            tc.tile_pool(name="psumB", bufs=4, space="PSUM") as ppB,
        ):
            # --- constants ---
            w1s = wp.tile([2 * CH[0], CH[0]], BF16)
            nc.sync.dma_start(out=w1s, in_=w1d[:, :])
            w2ps = wp.tile([2 * CH[0], 3 * CH[1]], BF16)
            nc.sync.dma_start(out=w2ps, in_=w2pd[:, :])
            w2cs = wp.tile([2 * CH[0], CH[1]], BF16)
            nc.sync.dma_start(out=w2cs, in_=w2cd[:, :])
            w2ss = wp.tile([2 * CH[0], CH[1]], BF16)
            nc.sync.dma_start(out=w2ss, in_=w2sd[:, :])
            w3s = wp.tile([CH[1], 9 * CH[2]], BF16)
            nc.sync.dma_start(out=w3s, in_=w3d[:, :])
            w4s = wp.tile([CH[2], 9 * CH[3]], BF16)
            nc.sync.dma_start(out=w4s, in_=w4d[:, :])
            b1s = wp.tile([CH[0], 1], F32)
            nc.sync.dma_start(out=b1s, in_=b1d[:, :])
            b2s = wp.tile([CH[1], 1], F32)
            nc.sync.dma_start(out=b2s, in_=b2d[:, :])
            b3s = wp.tile([CH[2], 1], F32)
            nc.sync.dma_start(out=b3s, in_=b3d[:, :])
            b4s = wp.tile([CH[3], 1], F32)
            nc.sync.dma_start(out=b4s, in_=b4d[:, :])
            pes = wp.tile([CH[3], T], F32)
            nc.sync.dma_start(out=pes, in_=ped[:, :])

            # --- persistent activation buffers, double-buffered ---
            sets = []
            for s in range(2):
                x9 = bp.tile([2 * CH[0], X9W], BF16,
                             name=f"x9_{s}")
                # h1: parts 0-63 = layer-1 output A (padded interior),
                # parts 64-127 = A shifted one column left (pairs layer
                # 2's (kh,0)+(kh,1) taps into K=128 matmuls)
                h1 = bp.tile([2 * CH[0], PAD], BF16, name=f"h1_{s}")
                # h1c: parts 0-63 = A again, parts 64-127 = A shifted
                # one row up (pairs taps (0,2)+(1,2))
                h1c = bp.tile([2 * CH[0], PAD], BF16, name=f"h1c_{s}")
                h2 = bp.tile([CH[1], PAD], BF16, name=f"h2_{s}")
                h3 = bp.tile([CH[2], PAD], BF16, name=f"h3_{s}")
                h4 = bp.tile([CH[3], NPIX], F32, name=f"h4_{s}")
                sets.append((x9, h1, h1c, h2, h3, h4))

            def x9_dma(img):
                x9 = sets[img % 2][0]
                qs = (nc.sync, nc.gpsimd)
                for kh in range(3):
                    for kw in range(3):
                        tap = kh * 3 + kw
                        base = tap * CIN
                        off = kh * PITCH + kw
                        qs[tap % 2].dma_start(
                            out=x9[base:base + CIN, 0:X9LEN],
                            in_=xin[img, :, off:off + X9LEN])
                nc.sync.dma_start(out=x9[CH[0]:CH[0] + K1, 0:X9LEN],
                                  in_=x9[0:K1, 0:X9LEN])

            def front(img):
                if img == 0:
                    x9_dma(0)
                if img + 1 < nimg:
                    x9_dma(img + 1)
                if img == 0:
                    # one-time halo zeroing, emitted after the first x9
                    # prefetches so DMA dispatch wins the gpsimd queue
                    for s in range(2):
                        for bi, buf in enumerate(sets[s][1:5]):
                            eng = (nc.vector if (bi + s) % 2 == 0
                                   else nc.gpsimd)
                            eng.memset(buf.bitcast(mybir.dt.uint16), 0.0)
                x9, h1, h1c, h2, h3, h4 = sets[img % 2]
                x9v = x9.rearrange("p (r c) -> p r c", c=PITCH)
                h1v = h1.rearrange("p (r c) -> p r c", c=PITCH)
                h1cv = h1c.rearrange("p (r c) -> p r c", c=PITCH)
                h2v = h2.rearrange("p (r c) -> p r c", c=PITCH)

                # layer 1: one K=45, M=64 matmul per tile; quads of
                # tiles run concurrently on the four 64x64 PE-array
                # quadrants (row groups use the x9 duplicate at
                # partitions 64-108), two tiles sharing each PSUM bank
                ps1 = {}
                for jq in range(0, NTILE, 4):
                    psa = ppA.tile([2 * CH[0], TILEPIX], F32,
                                   name=f"ps1_{img}_{jq}", tag="psA")
                    psb = ppA.tile([2 * CH[0], TILEPIX], F32,
                                   name=f"ps1_{img}_{jq + 2}", tag="psA")
                    for q in range(4):
                        j = jq + q
                        rq = (q // 2) * CH[0]      # array row offset
                        cq = (q % 2) * CH[0]       # array col offset
                        ps = (psa, psb)[q // 2]
                        nc.tensor.matmul(
                            ps[cq:cq + CH[0], :],
                            w1s[rq:rq + K1, :],
                            x9v[rq:rq + K1, j * RPT:(j + 1) * RPT, 0:W],
                            start=True, stop=True, tile_position=(rq, cq))
                        ps1[j] = ps[cq:cq + CH[0], :]
                for j in range(NTILE):
                    r0 = j * RPT
                    nc.scalar.activation(
                        h1v[0:CH[0], 1 + r0:1 + r0 + RPT, 1:1 + W], ps1[j],
                        AF.Lrelu, bias=b1s[:, 0:1], scale=1.0, alpha=ALPHA)
                    # flat contiguous shifted copies (halos are zero, so
                    # the wrap positions receive zeros - exact shifts):
                    s0 = (1 + r0) * PITCH
                    s1 = (1 + r0 + RPT) * PITCH
                    # A-colshift into h1 parts 64-127
                    nc.vector.tensor_copy(
                        h1[CH[0]:2 * CH[0], s0:s1],
                        h1[0:CH[0], s0 + 1:s1 + 1])
                    # A duplicate into h1c parts 0-63
                    nc.vector.tensor_copy(
                        h1c[0:CH[0], s0:s1], h1[0:CH[0], s0:s1])
                    # A-rowshift into h1c parts 64-127
                    nc.vector.tensor_copy(
                        h1c[CH[0]:2 * CH[0], s0 - PITCH:s1 - PITCH],
                        h1[0:CH[0], s0:s1])

                # layer 2: 4 paired + 1 single matmuls per tile, 2-tile
                # blocks with tap-outer order inside
                for jb in range(0, NTILE, BLOCKT):
                    jt = tuple(range(jb, jb + BLOCKT))
                    psb = [ppA.tile([CH[1], TILEPIX], F32,
                                    name=f"ps2_{img}_{j}", tag="psA")
                           for j in jt]
                    for kh in range(3):
                        for i, j in enumerate(jt):
                            r0 = j * RPT
                            nc.tensor.matmul(
                                psb[i], w2ps[:, kh * CH[1]:(kh + 1) * CH[1]],
                                h1v[:, r0 + kh:r0 + kh + RPT, 0:W],
                                start=(kh == 0), stop=False)
                    for i, j in enumerate(jt):
                        r0 = j * RPT
                        nc.tensor.matmul(
                            psb[i], w2cs,
                            h1cv[:, r0:r0 + RPT, 2:2 + W],
                            start=False, stop=False)
                    for i, j in enumerate(jt):
                        r0 = j * RPT
                        if i % 2 == 0:
                            nc.tensor.matmul(
                                psb[i], w2ss[0:CH[0], :],
                                h1v[0:CH[0], r0 + 2:r0 + 2 + RPT, 2:2 + W],
                                start=False, stop=True,
                                tile_position=(0, 0))
                        else:
                            # same tap via the B copy: B[., c] = A[., c+1],
                            # loaded at row group 2-3 so this matmul runs
                            # concurrently with its i%2==0 sibling
                            nc.tensor.matmul(
                                psb[i], w2ss[CH[0]:2 * CH[0], :],
                                h1v[CH[0]:2 * CH[0],
                                    r0 + 2:r0 + 2 + RPT, 1:1 + W],
                                start=False, stop=True,
                                tile_position=(CH[0], 0))
                    for i, j in enumerate(jt):
                        r0 = j * RPT
                        nc.scalar.activation(
                            h2v[:, 1 + r0:1 + r0 + RPT, 1:1 + W], psb[i],
                            AF.Lrelu, bias=b2s[:, 0:1], scale=1.0,
                            alpha=ALPHA)

            def back(img):
                t = img % T
                x9, h1, h1c, h2, h3, h4 = sets[img % 2]
                h2v = h2.rearrange("p (r c) -> p r c", c=PITCH)
                h3v = h3.rearrange("p (r c) -> p r c", c=PITCH)

                # layers 3-4: 9-tap shift-GEMM, 2-tile tap-outer blocks
                for li, (srcv, dstv, wsb, bsb, cout) in enumerate((
                    (h2v, h3v, w3s, b3s, CH[2]),
                    (h3v, None, w4s, b4s, CH[3]),
                )):
                    for jb in range(0, NTILE, BLOCKT):
                        jt = tuple(range(jb, jb + BLOCKT))
                        psb = [ppB.tile([cout, TILEPIX], F32,
                                       name=f"ps{li + 3}_{img}_{j}",
                                       tag="psB")
                               for j in jt]
                        for ti, (kh, kw) in enumerate(TAPS):
                            tap = kh * 3 + kw
                            for i, j in enumerate(jt):
                                r0 = j * RPT
                                nc.tensor.matmul(
                                    psb[i],
                                    wsb[:, tap * cout:(tap + 1) * cout],
                                    srcv[:, r0 + kh:r0 + kh + RPT, kw:kw + W],
                                    start=(ti == 0), stop=(ti == 8))
                        for i, j in enumerate(jt):
                            r0 = j * RPT
                            if dstv is not None:
                                nc.scalar.activation(
                                    dstv[:, 1 + r0:1 + r0 + RPT, 1:1 + W],
                                    psb[i], AF.Lrelu,
                                    bias=bsb[:, 0:1], scale=1.0, alpha=ALPHA)
                            else:
                                nc.scalar.activation(
                                    h4[:, j * TILEPIX:(j + 1) * TILEPIX],
                                    psb[i], AF.Lrelu,
                                    bias=bsb[:, 0:1], scale=1.0, alpha=ALPHA)

                # positional encoding add + store
                nc.vector.tensor_scalar_add(h4, h4, pes[:, t:t + 1])
                nc.sync.dma_start(out=outd[img], in_=h4)

            for step in range(nimg + 1):
                if step < nimg:
                    front(step)
                if step >= 1:
                    back(step - 1)

    nc.compile()
    return nc


def _pe_table():
    d = np.arange(CH[3])
    d_even = (d // 2) * 2
    tt = np.arange(T, dtype=np.float64)
    arg = tt[:, None] / np.power(10000.0, d_even / CH[3])
    pe = np.where(d % 2 == 0, np.sin(arg), np.cos(arg))  # [T, D]
    return np.ascontiguousarray(pe.T.astype(np.float32))  # [D, T]


def _cast_bf16(a):
    """fp32 -> bf16 (round-to-nearest-even) on the host so on-chip DMAs
    are plain copies."""
    import ml_dtypes
    return np.ascontiguousarray(np.asarray(a, dtype=np.float32)).astype(
        ml_dtypes.bfloat16)


def _w1_dup(w0):
    w1l = w0.transpose(2, 3, 1, 0).reshape(K1, CH[0])
    out = np.zeros((2 * CH[0], CH[0]), dtype=np.float32)
    out[0:K1] = w1l
    out[CH[0]:CH[0] + K1] = w1l
    return out


def _prep_consts(w0, b0, w1, b1, w2, b2, w3, b3):
    w0 = np.asarray(w0)
    a1 = np.asarray(w1).transpose(1, 2, 3, 0)  # [cin, kh, kw, cout]
    w2p = np.concatenate(
        [np.concatenate([a1[:, kh, 0, :], a1[:, kh, 1, :]], axis=0)
         for kh in range(3)], axis=1)          # [128, 3*128]
    w2c = np.concatenate([a1[:, 0, 2, :], a1[:, 1, 2, :]], axis=0)
    consts = {
        # [128, 64]: rows 0-44 and 64-108 both hold row (kh*3+kw)*5+cin
        "w1": _cast_bf16(_w1_dup(w0)),
        "w2p": _cast_bf16(w2p),
        "w2c": _cast_bf16(w2c),
        "w2s": _cast_bf16(
            np.concatenate([a1[:, 2, 2, :], a1[:, 2, 2, :]], axis=0)),
        "w3": _cast_bf16(
            np.asarray(w2).transpose(1, 2, 3, 0).reshape(CH[1], 9 * CH[2])),
        "w4": _cast_bf16(
            np.asarray(w3).transpose(1, 2, 3, 0).reshape(CH[2], 9 * CH[3])),
        "b1": np.ascontiguousarray(np.asarray(b0, dtype=np.float32)
                                   .reshape(CH[0], 1)),
        "b2": np.ascontiguousarray(np.asarray(b1, dtype=np.float32)
                                   .reshape(CH[1], 1)),
        "b3": np.ascontiguousarray(np.asarray(b2, dtype=np.float32)
                                   .reshape(CH[2], 1)),
        "b4": np.ascontiguousarray(np.asarray(b3, dtype=np.float32)
                                   .reshape(CH[3], 1)),
        "pe": _pe_table(),
    }
    return consts


_prog_cache: dict[int, object] = {}


def _get_program(nimg: int):
    if nimg not in _prog_cache:
        _prog_cache[nimg] = _build(nimg)
    return _prog_cache[nimg]


_runner_cache: dict[int, object] = {}


def _get_runner(nimg: int):
    """A reusable jitted SPMD executor for the per-core program (avoids
    re-tracing/re-lowering on every kernel() call)."""
    if nimg in _runner_cache:
        return _runner_cache[nimg]

    import jax
    import jax.numpy as jnp
    from concourse.bass2jax import (
        install_neuronx_cc_hook, partition_id_tensor, _bass_exec_p)
    from jax.sharding import Mesh, PartitionSpec, NamedSharding
    from jax.experimental.shard_map import shard_map

    nc = _get_program(nimg)
    install_neuronx_cc_hook()

    partition_name = (nc.partition_id_tensor.name
                      if nc.partition_id_tensor else None)
    in_names, out_names, out_avals, zero_shapes = [], [], [], []
    for alloc in nc.m.functions[0].allocations:
        if not isinstance(alloc, mybir.MemoryLocationSet):
            continue
        name = alloc.memorylocations[0].name
        if alloc.kind == "ExternalInput":
            if name != partition_name:
                in_names.append(name)
        elif alloc.kind == "ExternalOutput":
            shape = tuple(alloc.tensor_shape)
            dtype = mybir.dt.np(alloc.dtype)
            out_names.append(name)
            out_avals.append(jax.core.ShapedArray(shape, dtype))
            zero_shapes.append((shape, dtype))
    n_params = len(in_names)
    n_outs = len(out_names)
    all_in_names = list(in_names) + list(out_names)
    if partition_name is not None:
        all_in_names.append(partition_name)

    def _body(*args):
        operands = list(args)
        if partition_name is not None:
            operands.append(partition_id_tensor())
        outs = _bass_exec_p.bind(
            *operands,
            out_avals=tuple(out_avals),
            in_names=tuple(all_in_names),
            out_names=tuple(out_names),
            lowering_input_output_aliases=(),
            sim_require_finite=True,
            sim_require_nnan=True,
            nc=nc,
        )
        return tuple(outs)

    devices = jax.devices()[:N_CORES]
    mesh = Mesh(np.asarray(devices), ("core",))
    sh = NamedSharding(mesh, PartitionSpec("core"))
    donate = tuple(range(n_params, n_params + n_outs))
    sharded = jax.jit(
        shard_map(_body, mesh=mesh,
                  in_specs=(PartitionSpec("core"),) * (n_params + n_outs),
                  out_specs=(PartitionSpec("core"),) * n_outs,
                  check_rep=False),
        donate_argnums=donate, keep_unused=True)
    zeros_fn = jax.jit(
        lambda: tuple(
            jnp.zeros((N_CORES * s[0], *s[1:]), d) for s, d in zero_shapes),
        out_shardings=(sh,) * n_outs)

    def run(in_maps):
        concat_in = [
            np.concatenate([np.asarray(in_maps[c][nm])
                            for c in range(N_CORES)], axis=0)
            for nm in in_names
        ]
        dev_in = [jax.device_put(a, sh) for a in concat_in]
        outs = sharded(*dev_in, *zeros_fn())
        oi = out_names.index("out")
        return np.asarray(outs[oi])

    _runner_cache[nimg] = run
    return run


def make_in_maps(x, w0, b0, w1, b1, w2, b2, w3, b3):
    """Shard the full inputs into the 8 per-core input maps."""
    consts = _prep_consts(w0, b0, w1, b1, w2, b2, w3, b3)
    bpc = B // N_CORES  # batches per core
    nimg = bpc * T
    x = np.asarray(x)
    in_maps = []
    for c in range(N_CORES):
        xs = x[c * bpc:(c + 1) * bpc].reshape(nimg, CIN, H, W)
        xp = np.zeros((nimg, CIN, PITCH, PITCH), dtype=np.float32)
        xp[:, :, 1:1 + H, 1:1 + W] = xs
        in_maps.append(
            {"xin": _cast_bf16(xp.reshape(nimg, CIN, PAD)), **consts})
    return in_maps


def kernel(x, w0, b0, w1, b1, w2, b2, w3, b3):
    nimg = (B // N_CORES) * T
    run = _get_runner(nimg)
    in_maps = make_in_maps(x, w0, b0, w1, b1, w2, b2, w3, b3)
    glob = run(in_maps)  # [8*nimg, 128, 4096]
    bpc = B // N_CORES
    out = glob.reshape(N_CORES * bpc, T, CH[3], H, W).reshape(
        B, T, CH[3], H, W)
    return np.ascontiguousarray(out.astype(np.float32))
